# revision 22
# baseline (speedup 1.0000x reference)
"""Multi-head self-attention 2D (dense transformer) Bass kernel for Trainium2.

Problem: x [4, 512, 48, 48] fp32; qkv_w [1536, 512]; proj_w [512, 512].
  qkv 1x1-conv -> per-head attention (8 heads, head_dim 64) over N=2304
  spatial positions -> output projection.

Sharding (8 cores): core i handles batch b = i//2 and query half i%2
  (nq = 1152 queries). Each core computes K/V for the full batch image
  (keys/values need all N positions) and the final projection for its
  query columns, so per-core outputs are disjoint slices of the full
  output -- no collectives, gather on host.

Schedule: the ScalarE exp stream is the critical resource (exp must
  touch every score element and only ScalarE has Exp), so the kernel is
  organized to keep it saturated:
  - attention runs in 384-query chunks; each key-tile step emits
    S (fp16 scores, row-packed head pairs), exp (both heads, strided
    PSUM read), and one lagged AV per key-tile pair, so the in-order
    PE never waits on exp;
  - AV uses fp8e4m3 DoubleRow matmuls (keys packed 256-deep), halving
    PE time and giving it slack;
  - remaining phase-1 work (V projection, K/Q for later head pairs) and
    the per-pair output projection are "fillers" drained one per step
    into that PE slack, so ScalarE starts exp ~10us into the kernel and
    never starves afterwards;
  - V carries a ones-row so AV also accumulates softmax denominators;
    per-chunk normalize (reciprocal, ones-matmul broadcast, one 128-row
    multiply) lags a chunk behind.  Softmax max-subtraction is skipped:
    scores*scale ~ N(0,1) so exp stays in range.
"""

import numpy as np

B = 4
C = 512
HH = 48
WW = 48
N = HH * WW          # 2304
NQ = N // 2          # 1152 queries per core
HEADS = 8
D = C // HEADS       # 64
SCALE = float(D) ** -0.5
NCORES = 8

_CACHE: dict = {}


def _build_module(stage=4, loop_n=None, fp8_av=False):
    from collections import deque

    import concourse.mybir as mybir
    import concourse.tile as tile
    from concourse import bacc

    FP16 = mybir.dt.float16
    FP32 = mybir.dt.float32
    FP8 = mybir.dt.float8e4
    DR = mybir.MatmulPerfMode.DoubleRow
    AF = mybir.ActivationFunctionType
    EDT = FP8 if fp8_av else FP16

    nc = bacc.Bacc("TRN2", target_bir_lowering=False, debug=False)
    xk = nc.dram_tensor("xk", [C, N], FP16, kind="ExternalInput")
    xq = nc.dram_tensor("xq", [C, NQ], FP16, kind="ExternalInput")
    wqkv = nc.dram_tensor("wqkv", [C, 3 * C], FP16, kind="ExternalInput")
    wproj = nc.dram_tensor("wproj", [C, C], FP16, kind="ExternalInput")
    y = nc.dram_tensor("y", [C, NQ], FP32, kind="ExternalOutput")

    CT = C // 128     # 4 channel tiles
    MT = N // 128     # 18 key tiles
    MP = MT // 2      # 9 key-tile pairs
    CL = 384
    QCH = [(i * CL, CL) for i in range(NQ // CL)]    # 3 query chunks
    KCH = [(i * CL, CL) for i in range(N // CL)]     # 6 key chunks

    with tile.TileContext(nc) as tc:
        with (
            tc.tile_pool(name="consts", bufs=1) as cpool,
            tc.tile_pool(name="wts", bufs=1) as wpool,
            tc.tile_pool(name="qkv", bufs=1) as qkpool,
            tc.tile_pool(name="keep", bufs=1) as keep,
            tc.tile_pool(name="sps", bufs=2, space="PSUM") as spool,
            tc.tile_pool(name="avps", bufs=1, space="PSUM") as avps,
            tc.tile_pool(name="flex", bufs=2, space="PSUM") as flex,
            tc.tile_pool(name="esb", bufs=8) as epool,
            tc.tile_pool(name="avsb", bufs=2) as avpool,
            tc.tile_pool(name="p2sb", bufs=2) as p2sb,
        ):
            ones1 = cpool.tile([1, 64], FP16, name="ones1", tag="ones1")
            nc.vector.memset(ones1[:], 1.0)
            # exp bias: -2 keeps exp outputs inside fp8e4m3 range (max 448);
            # the constant shift cancels in the softmax normalization.
            ebias = cpool.tile([128, 1], FP32, name="ebias", tag="ebias")
            nc.vector.memset(ebias[:], -2.0)

            # input DMAs on two queues: x image via ActE queue, rest via SP
            wt = []
            wp = []
            xf = []
            xqt = []
            for t in range(CT):
                w = wpool.tile([128, 3 * C], FP16, name=f"w{t}", tag=f"w{t}")
                nc.sync.dma_start(w[:], wqkv.ap()[128 * t : 128 * (t + 1), :])
                wt.append(w)
                xt = keep.tile([128, N], FP16, name=f"x{t}", tag=f"x{t}")
                nc.scalar.dma_start(xt[:], xk.ap()[128 * t : 128 * (t + 1), :])
                xf.append(xt)
            for t in range(CT):
                xs = keep.tile([128, NQ], FP16, name=f"xq{t}", tag=f"xq{t}")
                nc.sync.dma_start(xs[:], xq.ap()[128 * t : 128 * (t + 1), :])
                xqt.append(xs)
            for t in range(CT):
                p = wpool.tile([128, C], FP16, name=f"wp{t}", tag=f"wp{t}")
                nc.sync.dma_start(p[:], wproj.ap()[128 * t : 128 * (t + 1), :])
                wp.append(p)

            qsb = [qkpool.tile([128, NQ], FP16, name=f"q{t}", tag=f"q{t}") for t in range(CT)]
            ksb = [qkpool.tile([128, N], FP16, name=f"k{t}", tag=f"k{t}") for t in range(CT)]
            # vT in key-pair-packed layout: [p, s, 65h+j] holds
            # vT[key 256*mp + 128*s + p, head h dim j]; col 64 of each head
            # block is the ones/denominator column.
            VW = 544 if fp8_av else 520
            vp = [qkpool.tile([128, 2, VW], EDT, name=f"v{mp}", tag=f"v{mp}") for mp in range(MP)]
            cssb = [
                [keep.tile([1, NQ], FP32, name=f"cs{t}_{h}", tag=f"cs{t}_{h}") for h in range(2)]
                for t in range(CT)
            ]
            oa = [keep.tile([128, NQ], FP16, name=f"oa{t}", tag=f"oa{t}") for t in range(CT)]
            oy = [keep.tile([128, NQ], FP32, name=f"oy{t}", tag=f"oy{t}") for t in range(CT)]

            for mp in range(MP):
                nc.gpsimd.memset(
                    vp[mp][:, :, 0:520].rearrange("p s (h w) -> p s h w", h=8)[:, :, :, 64:65],
                    1.0,
                )
                if fp8_av:
                    nc.gpsimd.memset(vp[mp][:, :, 520:544], 0.0)

            # ---- filler items (phase-1 leftovers + projection), each ----
            # ---- at most ~2 matmuls of PE work, drained 1 per step    ----
            def build_k_items(ct):
                items = []
                for n0, nl in KCH:
                    hold = {}
                    def i1(ct=ct, n0=n0, nl=nl, hold=hold):
                        ps = flex.tile([128, 512], FP32, name="psk", tag="flex")
                        hold["ps"] = ps
                        for kt in (0, 1):
                            nc.tensor.matmul(
                                ps[:, 0:nl],
                                lhsT=wt[kt][:, C + 128 * ct : C + 128 * (ct + 1)],
                                rhs=xf[kt][:, n0 : n0 + nl],
                                start=(kt == 0), stop=False,
                            )
                    def i2(ct=ct, n0=n0, nl=nl, hold=hold):
                        ps = hold.pop("ps")
                        for kt in (2, 3):
                            nc.tensor.matmul(
                                ps[:, 0:nl],
                                lhsT=wt[kt][:, C + 128 * ct : C + 128 * (ct + 1)],
                                rhs=xf[kt][:, n0 : n0 + nl],
                                start=False, stop=(kt == 3),
                            )
                        nc.vector.tensor_copy(ksb[ct][:, n0 : n0 + nl], ps[:, 0:nl])
                    items += [i1, i2]
                return items

            def build_q_items(ct):
                items = []
                for c0, cl in QCH:
                    hold = {}
                    def i1(ct=ct, c0=c0, cl=cl, hold=hold):
                        ps = flex.tile([128, 512], FP32, name="psq", tag="flex")
                        hold["ps"] = ps
                        for kt in (0, 1):
                            nc.tensor.matmul(
                                ps[:, 0:cl],
                                lhsT=wt[kt][:, 128 * ct : 128 * (ct + 1)],
                                rhs=xqt[kt][:, c0 : c0 + cl],
                                start=(kt == 0), stop=False,
                            )
                    def i2(ct=ct, c0=c0, cl=cl, hold=hold):
                        ps = hold.pop("ps")
                        for kt in (2, 3):
                            nc.tensor.matmul(
                                ps[:, 0:cl],
                                lhsT=wt[kt][:, 128 * ct : 128 * (ct + 1)],
                                rhs=xqt[kt][:, c0 : c0 + cl],
                                start=False, stop=(kt == 3),
                            )
                        nc.vector.tensor_copy(qsb[ct][:, c0 : c0 + cl], ps[:, 0:cl])
                    items += [i1, i2]
                return items

            def build_v_items():
                items = []
                for m in range(MT):
                    hold = {}
                    def i1(m=m, hold=hold):
                        ps = flex.tile([128, 512], FP32, name="psv", tag="flex")
                        hold["ps"] = ps
                        for kt in (0, 1):
                            nc.tensor.matmul(
                                ps[:, 0:512],
                                lhsT=xf[kt][:, 128 * m : 128 * (m + 1)],
                                rhs=wt[kt][:, 2 * C : 3 * C],
                                start=(kt == 0), stop=False,
                            )
                    def i2(m=m, hold=hold):
                        ps = hold.pop("ps")
                        for kt in (2, 3):
                            nc.tensor.matmul(
                                ps[:, 0:512],
                                lhsT=xf[kt][:, 128 * m : 128 * (m + 1)],
                                rhs=wt[kt][:, 2 * C : 3 * C],
                                start=False, stop=(kt == 3),
                            )
                        nc.vector.tensor_copy(
                            vp[m // 2][:, m % 2, 0:520].rearrange("p (h w) -> p h w", h=8)[:, :, 0:64],
                            ps[:].rearrange("p (h w) -> p h w", h=8),
                        )
                    items += [i1, i2]
                return items

            def build_proj_items(t, c0, cl):
                items = []
                for ct in range(CT):
                    def it(t=t, ct=ct, c0=c0, cl=cl):
                        ps = flex.tile([128, 512], FP32, name="py", tag="flex")
                        nc.tensor.matmul(
                            ps[:, 0:cl],
                            lhsT=wp[t][:, 128 * ct : 128 * (ct + 1)],
                            rhs=oa[t][:, c0 : c0 + cl],
                            start=True, stop=True,
                        )
                        if t == 0:
                            nc.vector.tensor_copy(oy[ct][:, c0 : c0 + cl], ps[:, 0:cl])
                        else:
                            nc.vector.tensor_add(
                                oy[ct][:, c0 : c0 + cl],
                                oy[ct][:, c0 : c0 + cl],
                                ps[:, 0:cl],
                            )
                        if t == CT - 1:
                            eng = nc.sync if ct % 2 == 0 else nc.scalar
                            eng.dma_start(
                                y.ap()[128 * ct : 128 * (ct + 1), c0 : c0 + cl],
                                oy[ct][:, c0 : c0 + cl],
                            )
                    items.append(it)
                return items

            # K chunks 0-1 and Q chunk 0 of head-pair 0 inline so the exp
            # stream starts as soon as possible; everything else is fillers.
            k0_items = build_k_items(0)
            q0_items = build_q_items(0)
            for it in k0_items[0:4]:
                it()
            for it in q0_items[0:2]:
                it()

            fillers = deque(k0_items[4:])
            fillers.extend(q0_items[2:])
            fillers.extend(build_v_items())
            for ct in range(1, CT):
                k = build_k_items(ct)
                q = build_q_items(ct)
                fillers.extend(
                    q[0:2] + k[0:6] + q[2:4] + k[6:10] + q[4:6] + k[10:12]
                )

            av_fifo = deque()
            tail_fifo = deque()

            def make_chunk(t, c0, cl):
                st = {"sp": {}, "es": {}, "av": None}
                kA = ksb[t][0:64, :]
                kB = ksb[t][64:128, :]
                qA = qsb[t][0:64, :]
                qB = qsb[t][64:128, :]

                def emit_S(m):
                    sp = spool.tile([128, 1024], FP32, name="s", tag="s")
                    ms = slice(128 * m, 128 * (m + 1))
                    nc.tensor.matmul(
                        sp[:, 0:cl], lhsT=kA[:, ms],
                        rhs=qA[:, c0 : c0 + cl],
                        start=True, stop=True, tile_position=(0, 0),
                    )
                    nc.tensor.matmul(
                        sp[:, 512 : 512 + cl], lhsT=kB[:, ms],
                        rhs=qB[:, c0 : c0 + cl],
                        start=True, stop=True, tile_position=(64, 0),
                    )
                    st["sp"][m] = sp

                def emit_E(m):
                    sp = st["sp"].pop(m)
                    if m % 2 == 0:
                        st["es"][m // 2] = epool.tile(
                            [128, 2, 2 * CL], EDT, name="es", tag="es"
                        )
                    es = st["es"][m // 2]
                    nc.scalar.activation(
                        es[:, m % 2, 0 : 2 * cl],
                        sp[:].rearrange("p (b c) -> p b c", b=2)[:, :, 0:cl],
                        AF.Exp, scale=SCALE, bias=ebias[:],
                    )

                def emit_tail(avs):
                    def run():
                        rec = p2sb.tile([1, 1024], FP16, name="rec", tag="rec")
                        with nc.allow_low_precision(reason="softmax recip fp16"):
                            nc.vector.reciprocal(
                                rec[:, 0:cl], cssb[t][0][:, c0 : c0 + cl]
                            )
                            nc.vector.reciprocal(
                                rec[:, 512 : 512 + cl], cssb[t][1][:, c0 : c0 + cl]
                            )
                        bc = flex.tile([128, 512], FP32, name="bc", tag="flex")
                        nc.tensor.matmul(
                            bc[0:64, 0:cl], lhsT=ones1[:],
                            rhs=rec[:, 0:cl], start=True, stop=True,
                        )
                        nc.tensor.matmul(
                            bc[64:128, 0:cl], lhsT=ones1[:],
                            rhs=rec[:, 512 : 512 + cl], start=True, stop=True,
                        )
                        nc.vector.tensor_mul(
                            oa[t][:, c0 : c0 + cl], avs[:, 0:cl], bc[:, 0:cl]
                        )
                    return run

                def emit_copies():
                    avA, avB = st["av"]
                    avs = avpool.tile([128, 512], FP16, name="avs", tag="avs")
                    nc.vector.tensor_copy(avs[0:64, 0:cl], avA[0:64, 0:cl])
                    nc.vector.tensor_copy(avs[64:128, 0:cl], avB[0:64, 0:cl])
                    nc.vector.tensor_copy(cssb[t][0][:, c0 : c0 + cl], avA[64:65, 0:cl])
                    nc.vector.tensor_copy(cssb[t][1][:, c0 : c0 + cl], avB[64:65, 0:cl])
                    tail_fifo.append((t, c0, cl, emit_tail(avs)))

                def emit_A(mp):
                    es = st["es"].pop(mp)
                    if st["av"] is None:
                        st["av"] = (
                            avps.tile([65, 512], FP32, name="avA", tag="avA"),
                            avps.tile([65, 512], FP32, name="avB", tag="avB"),
                        )
                    avA, avB = st["av"]
                    if fp8_av:
                        nc.tensor.matmul(
                            avA[:, 0:cl],
                            lhsT=vp[mp][:, :, 130 * t : 130 * t + 65],
                            rhs=es[:, :, 0:cl],
                            start=(mp == 0), stop=(mp == MP - 1),
                            perf_mode=DR,
                        )
                        nc.tensor.matmul(
                            avB[:, 0:cl],
                            lhsT=vp[mp][:, :, 130 * t + 65 : 130 * t + 130],
                            rhs=es[:, :, cl : 2 * cl],
                            start=(mp == 0), stop=(mp == MP - 1),
                            perf_mode=DR,
                        )
                    else:
                        for s in range(2):
                            nc.tensor.matmul(
                                avA[:, 0:cl],
                                lhsT=vp[mp][:, s, 130 * t : 130 * t + 65],
                                rhs=es[:, s, 0:cl],
                                start=(mp == 0 and s == 0),
                                stop=(mp == MP - 1 and s == 1),
                            )
                            nc.tensor.matmul(
                                avB[:, 0:cl],
                                lhsT=vp[mp][:, s, 130 * t + 65 : 130 * t + 130],
                                rhs=es[:, s, cl : 2 * cl],
                                start=(mp == 0 and s == 0),
                                stop=(mp == MP - 1 and s == 1),
                            )
                    if mp == MP - 1:
                        emit_copies()

                return emit_S, emit_E, emit_A

            def pop_tail():
                tt, tc0, tcl, run = tail_fifo.popleft()
                run()
                fillers.extend(build_proj_items(tt, tc0, tcl))

            def _run_phases():
                drained = 0
                for t in range(CT):
                    for c0, cl in QCH:
                        eS, eE, eA = make_chunk(t, c0, cl)
                        for m in range(MT):
                            eS(m)
                            eE(m)
                            for _ in range(2 if drained < 48 else 1):
                                if fillers:
                                    fillers.popleft()()
                                    drained += 1
                            if m % 2 == 1:
                                av_fifo.append(lambda mp=m // 2, f=eA: f(mp))
                                if len(av_fifo) > 5:
                                    av_fifo.popleft()()
                            if m == 4 and tail_fifo:
                                pop_tail()
                while av_fifo:
                    av_fifo.popleft()()
                while tail_fifo:
                    pop_tail()
                while fillers:
                    fillers.popleft()()

            import contextlib
            loop_ctx = tc.For_i(0, loop_n, 1) if loop_n else contextlib.nullcontext()
            with loop_ctx:
                _run_phases()

    nc.compile()
    return nc


def _get_module():
    if "nc" not in _CACHE:
        _CACHE["nc"] = _build_module()
    return _CACHE["nc"]


def make_in_maps(x, qkv_w, proj_w):
    xf = np.asarray(x, dtype=np.float32).reshape(B, C, N)
    wq = np.ascontiguousarray(np.asarray(qkv_w).T).astype(np.float16)
    wpj = np.ascontiguousarray(np.asarray(proj_w).T).astype(np.float16)
    in_maps = []
    for i in range(NCORES):
        b, h = divmod(i, 2)
        xkc = np.ascontiguousarray(xf[b]).astype(np.float16)
        xqc = np.ascontiguousarray(xf[b][:, h * NQ : (h + 1) * NQ]).astype(np.float16)
        in_maps.append({"xk": xkc, "xq": xqc, "wqkv": wq, "wproj": wpj})
    return in_maps


def gather_out(results):
    out = np.empty((B, C, N), np.float32)
    for i in range(NCORES):
        b, h = divmod(i, 2)
        out[b][:, h * NQ : (h + 1) * NQ] = results[i]["y"]
    return out.reshape(B, C, HH, WW)


def kernel(x, qkv_w, proj_w):
    from concourse import bass_utils

    nc = _get_module()
    in_maps = make_in_maps(x, qkv_w, proj_w)
    res = bass_utils.run_bass_kernel_spmd(
        nc, in_maps, core_ids=list(range(NCORES)), trace=False
    )
    return gather_out(res.results)
